# revision 18
# baseline (speedup 1.0000x reference)
"""MoE layer (B=2,T=2048,D=1024,H=4096,E=8,top-2) on 8 Trainium2 NeuronCores.

Expert-parallel: core e owns expert e's weights.  Each core:
  1. computes the router logits in true fp32 on-device (logitsT = wg.T @ xT via
     the PE, then PE-transpose to token-major layout),
  2. softmax + top-2 selection (exact fp32 comparisons against the row max8),
     producing the per-token coefficient for *its* expert:
         c[t] = probs[t,0]*[top1(t)==e] + probs[t,1]*[top2(t)==e]
     (bug-faithful slot weighting from the reference),
  3. runs the expert MLP in bf16 (w1/w2 pre-cast on host) with gelu+bias fused
     into the PSUM eviction of GEMM1 and (y+b2)*c fused into GEMM2's eviction,
  4. DENSE mode: all 4096 tokens flow through every expert (reference-style
     grouped GEMM); contrib_e = c ⊙ MLP_e(x) is written densely.
     SPARSE mode (default): tokens are compacted on-device (prefix-sum via
     cumulative adds + a triangular-ones matmul, then one dma_scatter_add
     builds the slot table), gathered+transposed straight into SBUF by a
     single Q7 dma_gather per n-tile, and the per-slot outputs are
     scatter-added into the output rows; pad slots target a trash row so no
     runtime count register is needed.  CAP=1152 per expert (~4.5 sigma over
     the balanced load of 1024).
Host: shards/pre-transposes inputs, sums the 8 per-core contributions.
"""

import os
from contextlib import ExitStack

import numpy as np
import ml_dtypes

import concourse.bass as bass
import concourse.mybir as mybir
import concourse.tile as tile
from concourse import bacc
from concourse.bass import ds, ts
from concourse.masks import make_identity, make_upper_triangular
from concourse.kernels.tile_matmul import (
    composable_matmul_tile_kernel,
    dma_from_dram_kxm,
    dma_from_dram_kxn,
    dma_to_dram_mxn,
    k_pool_min_bufs,
)
from concourse import bass_utils

P = 128
B, TT, D = 2, 2048, 1024
T = B * TT            # 4096 tokens
H = 4 * D             # 4096
E = 8                 # experts == cores
TO = T // P           # 32 token tile-columns
KD = D // P           # 8 k-subtiles over D
f32 = mybir.dt.float32
bf16 = mybir.dt.bfloat16
i32 = mybir.dt.int32
i16 = mybir.dt.int16
AF = mybir.ActivationFunctionType
ALU = mybir.AluOpType

SPARSE = os.environ.get("MOE_SPARSE", "1") == "1"
CAP = int(os.environ.get("MOE_CAP", "1152"))  # per-expert token capacity (sparse)
CAPO = CAP // P                               # slot tile-columns


def _build_routing(nc, tc, ctx, const, xT32, wg_ap, logits_out):
    """Router block: fp32 logits + softmax + top2 one-hots -> c [P, TO].

    Returns (c, assigned) SBUF tiles, both [P, TO] f32, token t = o*128 + p.
    """
    ident = const.tile([P, P], f32)
    make_identity(nc, ident)

    wg_sb = const.tile([P, KD, E], f32)
    nc.sync.dma_start(wg_sb[:], wg_ap.rearrange("(ko ki) e -> ki ko e", ki=P))

    ebc = const.tile([P, E], f32)  # one-hot of my expert, bcast over partitions
    evec_sb = const.tile([1, E], f32)
    nc.sync.dma_start(evec_sb[:], nc.evec_ap)
    nc.gpsimd.partition_broadcast(ebc[:], evec_sb[:])

    lg = const.tile([P, TO, E], f32)    # logits, token-major

    with (
        tc.tile_pool(name="rpool", bufs=3) as rpool,
        tc.tile_pool(name="rpsum", bufs=2, space="PSUM") as rpsum,
    ):
        NT = 512
        xTr = xT32.rearrange("(ko ki) t -> ki ko t", ki=P)
        for n in range(T // NT):
            xt_t = rpool.tile([P, KD, NT], f32, tag="xt")
            nc.sync.dma_start(xt_t[:], xTr[:, :, ds(n * NT, NT)])
            ps = rpsum.tile([E, NT], f32, tag="lgps")
            for k in range(KD):
                nc.tensor.matmul(
                    ps[:], wg_sb[:, k, :], xt_t[:, k, :],
                    start=(k == 0), stop=(k == KD - 1),
                )
            lt = rpool.tile([E, NT], f32, tag="lt")
            nc.vector.tensor_copy(lt[:], ps[:])
            for j in range(NT // P):
                pst = rpsum.tile([P, E], f32, tag="pst")
                nc.tensor.transpose(pst[:], lt[:, ts(j, P)], ident[:E, :E])
                nc.vector.tensor_copy(lg[:, n * (NT // P) + j, :], pst[:])

        # logits output (all cores compute it; host reads core 0's)
        nc.sync.dma_start(logits_out.rearrange("(o p) e -> p o e", p=P), lg[:])

        # softmax pieces.  |logits| < ~3 so exp without max-subtraction is safe.
        ex = rpool.tile([P, TO, E], f32, tag="ex")
        nc.scalar.activation(ex[:], lg[:], AF.Exp)
        den = rpool.tile([P, TO], f32, tag="den")
        nc.vector.tensor_reduce(
            out=den[:], in_=ex[:], axis=mybir.AxisListType.X, op=ALU.add
        )
        rden = rpool.tile([P, TO], f32, tag="rden")
        nc.vector.reciprocal(rden[:], den[:])
        p0 = rpool.tile([P, TO], f32, tag="p0")
        p1 = rpool.tile([P, TO], f32, tag="p1")
        nc.vector.tensor_mul(p0[:], ex[:, :, 0], rden[:])
        nc.vector.tensor_mul(p1[:], ex[:, :, 1], rden[:])

        # top-2 on exact fp32 logits.  tensor_tensor comparisons and
        # tensor_tensor_reduce crash on this runtime, so: `le` (my expert's
        # logit) via one-hot mult + reduce, equality via the exact
        # 1 - min((a-b)^2 * 1e38, 1) trick (gaps between distinct logits are
        # >=1e-7 for this distribution, so the product saturates).
        mx8 = rpool.tile([P, TO, 8], f32, tag="mx8")
        le = rpool.tile([P, TO], f32, tag="le")
        for o in range(TO):
            nc.vector.max(mx8[:, o, :], lg[:, o, :])
            scr = rpool.tile([P, E], f32, tag="scr")
            nc.vector.tensor_mul(scr[:], lg[:, o, :], ebc[:])
            nc.vector.tensor_reduce(
                out=le[:, o : o + 1], in_=scr[:], axis=mybir.AxisListType.X,
                op=ALU.add,
            )

        def eq_onehot(out_t, a, b, tagp):
            d = rpool.tile([P, TO], f32, tag=f"{tagp}_d")
            nc.vector.tensor_sub(d[:], a, b)
            nc.vector.tensor_mul(d[:], d[:], d[:])
            nc.vector.tensor_scalar(d[:], d[:], 1e38, 1.0, op0=ALU.mult, op1=ALU.min)
            nc.vector.tensor_scalar(out_t[:], d[:], -1.0, 1.0, op0=ALU.mult, op1=ALU.add)

        is1 = rpool.tile([P, TO], f32, tag="is1")
        is2 = rpool.tile([P, TO], f32, tag="is2")
        eq_onehot(is1, le[:], mx8[:, :, 0], "e1")
        eq_onehot(is2, le[:], mx8[:, :, 1], "e2")

        c = const.tile([P, TO], f32)
        asn = const.tile([P, TO], f32)
        t0 = rpool.tile([P, TO], f32, tag="t0")
        t1 = rpool.tile([P, TO], f32, tag="t1")
        nc.vector.tensor_mul(t0[:], p0[:], is1[:])
        nc.vector.tensor_mul(t1[:], p1[:], is2[:])
        nc.vector.tensor_add(c[:], t0[:], t1[:])
        nc.vector.tensor_add(asn[:], is1[:], is2[:])
    return c, asn


def build_moe_nc(sparse: bool):
    nc = bacc.Bacc("TRN2", target_bir_lowering=False, debug=False, num_swdge_queues=4)

    xT32 = nc.dram_tensor("xT32", [D, T], f32, kind="ExternalInput").ap()
    wg_ap = nc.dram_tensor("wg", [D, E], f32, kind="ExternalInput").ap()
    w1s = nc.dram_tensor("w1s", [D, H], bf16, kind="ExternalInput").ap()
    b1s = nc.dram_tensor("b1s", [P, H // P], f32, kind="ExternalInput").ap()
    w2s = nc.dram_tensor("w2s", [H, D], bf16, kind="ExternalInput").ap()
    b2s = nc.dram_tensor("b2s", [1, D], f32, kind="ExternalInput").ap()
    evec = nc.dram_tensor("evec", [1, E], f32, kind="ExternalInput").ap()
    nc.evec_ap = evec
    if sparse:
        xpad = nc.dram_tensor("xpad", [T + 1, D], bf16, kind="ExternalInput").ap()
        contrib = nc.dram_tensor(
            "contrib", [T + 1, D], f32, kind="ExternalOutput"
        ).ap()
    else:
        xTb = nc.dram_tensor("xTb", [D, T], bf16, kind="ExternalInput").ap()
        contrib = nc.dram_tensor("contrib", [T, D], f32, kind="ExternalOutput").ap()
    logits_out = nc.dram_tensor("logits_out", [T, E], f32, kind="ExternalOutput").ap()

    with tile.TileContext(nc) as tc, ExitStack() as ctx:
        const = ctx.enter_context(tc.tile_pool(name="const", bufs=1))

        b1_sb = const.tile([P, H // P], f32)
        nc.sync.dma_start(b1_sb[:], b1s)
        b2row = const.tile([1, D], f32)
        nc.sync.dma_start(b2row[:], b2s)
        b2bc = const.tile([P, D], f32)
        nc.gpsimd.partition_broadcast(b2bc[:], b2row[:])

        c, asn = _build_routing(nc, tc, ctx, const, xT32, wg_ap, logits_out)

        dram = ctx.enter_context(tc.tile_pool(name="dram", bufs=1, space="DRAM"))

        if sparse:
            _build_sparse_mlp(
                nc, tc, ctx, const, dram, c, asn, xpad, w1s, w2s, b1_sb, b2bc,
                contrib,
            )
        else:
            _build_dense_mlp(
                nc, tc, ctx, const, dram, c, xTb, w1s, w2s, b1_sb, b2bc, contrib
            )

    nc.compile()
    return nc


def _build_dense_mlp(nc, tc, ctx, const, dram, c, xTb, w1s, w2s, b1_sb, b2bc, contrib):
    hT = dram.tile([H, T], bf16)

    # GEMM1: hT[h,t] = gelu(sum_d w1[d,h] * x[t,d] + b1[h])
    with (
        tc.tile_pool(name="g1kxm", bufs=k_pool_min_bufs(w1s)) as g1m,
        tc.tile_pool(name="g1kxn", bufs=k_pool_min_bufs(xTb)) as g1n,
    ):
        kxm_prod, kxm_shape = dma_from_dram_kxm(g1m, w1s)
        kxn_prod, kxn_shape = dma_from_dram_kxn(g1n, xTb)

        def g1_red(nc, psum, sbuf, md):
            col = md.m_tile_idx * md.m_subtiles + md.m_subtile_idx
            nc.scalar.activation(sbuf, psum, AF.Gelu, bias=b1_sb[:, col : col + 1])

        composable_matmul_tile_kernel(
            tc=tc,
            kxm_shape=kxm_shape,
            kxn_shape=kxn_shape,
            output_type=bf16,
            kxm_producer=kxm_prod,
            kxn_producer=kxn_prod,
            mxn_consumer=dma_to_dram_mxn(hT[:]),
            mxn_subtile_reducer=g1_red,
        )

    # GEMM2: contrib[t,d] = c[t] * (sum_h hT[h,t] * w2[h,d] + b2[d])
    with (
        tc.tile_pool(name="g2kxm", bufs=k_pool_min_bufs(hT[:])) as g2m,
        tc.tile_pool(name="g2kxn", bufs=k_pool_min_bufs(w2s)) as g2n,
    ):
        kxm_prod2, kxm_shape2 = dma_from_dram_kxm(g2m, hT[:])
        kxn_prod2, kxn_shape2 = dma_from_dram_kxn(g2n, w2s)

        def g2_red(nc, psum, sbuf, md):
            col = md.m_tile_idx * md.m_subtiles + md.m_subtile_idx
            nstart = md.n_tile_idx * md.n_tile + md.n_subtile_idx * md.n_subtile
            nsz = psum.shape[-1]
            nc.vector.tensor_add(sbuf, psum, b2bc[:, ds(nstart, nsz)])
            nc.vector.tensor_scalar_mul(sbuf, sbuf, c[:, col : col + 1])

        composable_matmul_tile_kernel(
            tc=tc,
            kxm_shape=kxm_shape2,
            kxn_shape=kxn_shape2,
            output_type=f32,
            kxm_producer=kxm_prod2,
            kxn_producer=kxn_prod2,
            mxn_consumer=dma_to_dram_mxn(contrib),
            mxn_subtile_reducer=g2_red,
        )


def _build_sparse_mlp(
    nc, tc, ctx, const, dram, c, asn, xpadb, w1s, w2s, b1_sb, b2bc, contrib
):
    """Capacity-based sparse dispatch via the Q7 DMA gather/scatter family.

    Slot order is partition-major over the [P, TO] token grid.  A single
    dma_scatter_add builds the slot table (col0 = token id, col1 = c) from a
    pos-indexed scatter of all 4096 tokens; unassigned tokens land in trash
    rows >= count.  Pad slots in [count, CAP) keep the dump token T: the
    x-gather then reads xpadb's zero row and the y-scatter adds into
    contrib's trash row, so no runtime count register is needed anywhere.
    """
    CAPO16 = CAP // 16

    sut = const.tile([P, P], f32)  # sut[q,p] = 1 iff q < p (strict upper)
    make_upper_triangular(nc, sut[:], val=1.0, diag=False)

    tbl = dram.tile([T + 1, 64], f32)   # col0 = token id, col1 = c
    posd = dram.tile([T, 1], f32)

    xg_all = const.tile([P, CAP // 384, KD, 384], bf16)  # per-n-tile gather blocks
    y_all = const.tile([P, CAPO, D], f32)         # GEMM2 output rows
    idx16b = const.tile([P, CAPO16], i16)
    pos16b = const.tile([P, T // 16], i16)
    cslot = const.tile([P, CAPO], f32)

    with (
        tc.tile_pool(name="spool", bufs=2) as sp,
        tc.tile_pool(name="spsum", bufs=2, space="PSUM") as spsum,
    ):
        # ---- table init: col0 = T (dump token), col1.. = 0, rows [0, T)
        zt = sp.tile([P, 8, 64], f32, name="zt")
        nc.vector.memset(zt[:], 0.0)
        nc.vector.memset(zt[:, :, 0:1], float(T))
        tblr = tbl[:T].rearrange("(o p) k -> p o k", p=P)
        for o8 in range(TO // 8):
            nc.sync.dma_start(tblr[:, ds(o8 * 8, 8), :], zt[:])

        # ---- slot position per token (pad/unassigned -> T)
        cum = [sp.tile([P, TO], f32, name=f"cum{i}") for i in range(2)]
        src_t = asn
        dst = cum[0]
        sh = 1
        while sh < TO:
            nc.vector.tensor_add(dst[:, sh:], src_t[:, sh:], src_t[:, : TO - sh])
            nc.vector.tensor_copy(dst[:, :sh], src_t[:, :sh])
            src_t, dst = dst, (cum[1] if dst is cum[0] else cum[0])
            sh *= 2
        incl = src_t
        rtot = sp.tile([P, 1], f32, name="rtot")
        nc.vector.tensor_copy(rtot[:], incl[:, TO - 1 : TO])
        eps = spsum.tile([P, 1], f32, name="eps")
        nc.tensor.matmul(eps[:], sut[:], rtot[:], start=True, stop=True)
        excl = sp.tile([P, 1], f32, name="excl")
        nc.vector.tensor_copy(excl[:], eps[:])

        pos = sp.tile([P, TO], f32, name="pos")
        nc.vector.tensor_sub(pos[:], incl[:], asn[:])
        nc.vector.tensor_add(pos[:], pos[:], excl[:].to_broadcast([P, TO]))
        nc.vector.tensor_mul(pos[:], pos[:], asn[:])
        nd = sp.tile([P, TO], f32, name="nd")
        nc.vector.tensor_scalar(nd[:], asn[:], -float(T), float(T),
                                op0=ALU.mult, op1=ALU.add)
        nc.vector.tensor_add(pos[:], pos[:], nd[:])
        nc.sync.dma_start(posd[:].rearrange("(o p) one -> p (o one)", p=P), pos[:])

        # pos16b[b, s16] = pos of token s16*16 + (b % 16), int16, 8x replicated
        pos16f = sp.tile([P, T // 16], f32, name="pos16f")
        pview = posd[:].rearrange("(s16 b) one -> b (s16 one)", b=16)
        for a in range(8):
            nc.sync.dma_start(pos16f[ds(a * 16, 16), :], pview)
        nc.vector.tensor_copy(pos16b[:], pos16f[:])

        # ---- payload: col0 = token_id - T, col1 = c
        toki = sp.tile([P, TO], i32, name="toki")
        nc.gpsimd.iota(toki[:], pattern=[[P, TO]], base=-T, channel_multiplier=1)
        pay = sp.tile([P, TO, 64], f32, name="pay")
        nc.vector.memset(pay[:], 0.0)
        nc.vector.tensor_copy(pay[:, :, 0], toki[:])
        nc.vector.tensor_copy(pay[:, :, 1], c[:])

        nc.gpsimd.dma_scatter_add(tbl[:], pay[:], pos16b[:16, :], T, T, 64)

        # ---- read back compacted idx / c
        idxf = sp.tile([P, CAPO16], f32, name="idxf")
        iview = tbl[:CAP, 0:1].rearrange("(s16 b) one -> b (s16 one)", b=16)
        for a in range(8):
            nc.sync.dma_start(idxf[ds(a * 16, 16), :], iview)
        nc.vector.tensor_copy(idx16b[:], idxf[:])
        nc.sync.dma_start(
            cslot[:], tbl[:CAP, 1:2].rearrange("(j q) one -> q (j one)", q=P)
        )

        # ---- gather tokens as xT tiles (transpose gather), per-n-tile blocks
        for i in range(CAP // 384):
            nc.gpsimd.dma_gather(
                xg_all[:, i], xpadb, idx16b[:16, ds(24 * i, 24)],
                384, 384, D, elem_step=D, transpose=True, queue_num=i % 4,
            )

    # hT split into 16 parts of 256 h-rows so GEMM2 k-tiles can start as soon
    # as their rows exist (per-part dependency tracking -> GEMM1/GEMM2 overlap)
    G1_MT = 256
    NPART = H // G1_MT
    hT_parts = [dram.tile([G1_MT, CAP], bf16, name=f"hTp{m}") for m in range(NPART)]

    from concourse.kernels.tile_matmul import ShapeInfo

    g1m = ctx.enter_context(tc.tile_pool(name="g1kxm", bufs=k_pool_min_bufs(w1s)))
    g2m = ctx.enter_context(tc.tile_pool(name="g2kxm", bufs=4))
    g2n = ctx.enter_context(tc.tile_pool(name="g2kxn", bufs=3))
    if True:
        kxm_prod, kxm_shape = dma_from_dram_kxm(g1m, w1s)
        kxn_shape = ShapeInfo(pdims=((P, KD),), fdims=(CAP,))

        def g1_kxn_producer(nc, md):
            return xg_all[:, md.n_tile_idx, ts(md.k_tile_idx, md.k_subtiles), :]

        def g1_red(nc, psum, sbuf, md):
            col = md.m_tile_idx * md.m_subtiles + md.m_subtile_idx
            nc.scalar.activation(sbuf, psum, AF.Gelu, bias=b1_sb[:, col : col + 1])

        def g1_consumer(nc, mxn_tile, md):
            part = hT_parts[md.m_tile_idx]
            nsz = min(md.n_tile, CAP - md.n_tile_idx * md.n_tile)
            nc.sync.dma_start(
                part[:].rearrange("(ko ki) s -> ki ko s", ki=P)[
                    :, :, ds(md.n_tile_idx * md.n_tile, nsz)
                ],
                mxn_tile[:, :, :nsz],
            )

        composable_matmul_tile_kernel(
            tc=tc,
            kxm_shape=kxm_shape,
            kxn_shape=kxn_shape,
            output_type=bf16,
            kxm_producer=kxm_prod,
            kxn_producer=g1_kxn_producer,
            mxn_consumer=g1_consumer,
            mxn_subtile_reducer=g1_red,
            MAX_TILE_SIZE=384,
            MATMUL_FREE_DIM=384,
        )

    # GEMM2: y_all[s, d] = cslot[s] * (sum_h hT[h, s] * w2[h, d] + b2[d]);
    # per-subtile dma_scatter_add into contrib rows (pads hit the trash row).
    # (pools opened above so both GEMMs can interleave)
    if True:
        kxm_shape2 = ShapeInfo(pdims=((P, H // P),), fdims=(CAP,))

        def kxm_prod2(nc, md):
            t = g2m.tile([P, md.k_subtiles, md.m_tile], bf16, tag="g2kxm_t")
            parts_per_ktile = md.k_tile // G1_MT
            sub_per_part = G1_MT // P
            for i in range(parts_per_ktile):
                part = hT_parts[md.k_tile_idx * parts_per_ktile + i]
                nc.sync.dma_start(
                    t[:, ds(i * sub_per_part, sub_per_part), :],
                    part[:].rearrange("(ko ki) s -> ki ko s", ki=P)[
                        :, :, ds(md.m_tile_idx * md.m_tile, md.m_tile)
                    ],
                )
            return t[:]

        kxn_prod2, kxn_shape2 = dma_from_dram_kxn(g2n, w2s)

        def g2_producer(nc, md):
            return y_all[:, ts(md.m_tile_idx, md.m_subtiles), :]

        def g2_red(nc, psum, sbuf, md):
            col = md.m_tile_idx * md.m_subtiles + md.m_subtile_idx
            nstart = md.n_tile_idx * md.n_tile + md.n_subtile_idx * md.n_subtile
            nsz = psum.shape[-1]
            nc.vector.tensor_add(sbuf, psum, b2bc[:, ds(nstart, nsz)])
            nc.vector.tensor_scalar_mul(sbuf, sbuf, cslot[:, col : col + 1])

        def g2_consumer(nc, mxn_tile, md):
            mt = md.m_tile_idx
            for mi in range(md.m_subtiles):
                col = mt * md.m_subtiles + mi
                nc.gpsimd.dma_scatter_add(
                    contrib[:], mxn_tile[:, mi : mi + 1, :],
                    idx16b[:16, ds(col * 8, 8)],
                    P, P, D, queue_num=(col % 3) + 1,
                )

        composable_matmul_tile_kernel(
            tc=tc,
            kxm_shape=kxm_shape2,
            kxn_shape=kxn_shape2,
            output_type=None,
            kxm_producer=kxm_prod2,
            kxn_producer=kxn_prod2,
            mxn_consumer=g2_consumer,
            mxn_subtile_reducer=g2_red,
            mxn_subtile_producer=g2_producer,
            MAX_TILE_SIZE=1024,
            cache_tiles=False,
        )


# ---------------------------------------------------------------- host side

_CACHE = {}


def _get_nc(sparse):
    key = ("sparse" if sparse else "dense",)
    if key not in _CACHE:
        _CACHE[key] = build_moe_nc(sparse)
    return _CACHE[key]


def kernel(x, wg, w1, b1, w2, b2, _want_perf=False, _sparse=None):
    sparse = SPARSE if _sparse is None else _sparse
    x = np.asarray(x)
    wg = np.asarray(wg)
    w1 = np.asarray(w1)
    b1 = np.asarray(b1)
    w2 = np.asarray(w2)
    b2 = np.asarray(b2)

    xt = np.ascontiguousarray(x.reshape(T, D).astype(np.float32))
    xT32 = np.ascontiguousarray(xt.T)
    wg32 = np.ascontiguousarray(wg.astype(np.float32))

    shared = {"xT32": xT32, "wg": wg32}
    if sparse:
        xpad = np.zeros((T + 1, D), ml_dtypes.bfloat16)
        xpad[:T] = xt.astype(ml_dtypes.bfloat16)
        shared["xpad"] = xpad
    else:
        shared["xTb"] = np.ascontiguousarray(xT32.astype(ml_dtypes.bfloat16))

    in_maps = []
    for e in range(E):
        evec = np.zeros((1, E), np.float32)
        evec[0, e] = 1.0
        m = dict(shared)
        m["w1s"] = np.ascontiguousarray(w1[e].astype(ml_dtypes.bfloat16))
        m["b1s"] = np.ascontiguousarray(
            b1[e].astype(np.float32).reshape(H // P, P).T
        )
        m["w2s"] = np.ascontiguousarray(w2[e].astype(ml_dtypes.bfloat16))
        m["b2s"] = np.ascontiguousarray(b2[e].astype(np.float32).reshape(1, D))
        m["evec"] = evec
        in_maps.append(m)

    nc = _get_nc(sparse)
    res = bass_utils.run_bass_kernel_spmd(nc, in_maps, core_ids=list(range(E)))

    final = np.zeros((T, D), np.float32)
    for e in range(E):
        ce = res.results[e]["contrib"]
        final += ce[:T]
    logits = res.results[0]["logits_out"]

    out = (final.reshape(B, TT, D), logits)
    if _want_perf:
        return out, res
    return out
